# revision 11
# baseline (speedup 1.0000x reference)
"""Trainium2 Bass kernel for nn_AttentionBlock (GroupNorm + single-head spatial
self-attention + residual) on 8 NeuronCores.

Sharding: data-parallel over batch (2) x sequence-parallel over the query
dimension (4 chunks of 1024 of the 4096 spatial tokens). Each core gets the
full image of its batch element, ROTATED so its query chunk sits at token 0
(GroupNorm stats, key/value sets and softmax sums are permutation-invariant
over tokens, so rotation lets all 8 cores run the identical SPMD program).

Per-core dataflow (all channel-major [C on partitions] except v):
  phase 1: GroupNorm stats via bn_stats/bn_aggr per channel, group-combine via
           a tiny PE matmul with a 1/16 block indicator, broadcast back with a
           second indicator matmul -> per-channel Scale/Bias.
  phase 2: stream x in 512-token chunks, hn = x*Scale+Bias, project
           k [C, 4096], vT [4096, C] (transposed layout so the later AV matmul
           needs no transposes), q [C, 1024] (first two chunks = query tokens).
  phase 3: per 512-query half: scores^T [j,128, i,512] = k_tile^T @ q (PSUM
           accum over C), exp on ScalarE straight out of PSUM (no max
           subtraction -- logits are O(5)), row-sums r via a ones-vector
           matmul, AV accum hattn[c, i] += vT_tile^T @ p without any
           transposes, deferred softmax normalization folded into the
           PSUM->SBUF move, then the output projection + bias + residual.

Matmuls run as float32r (full fp32 storage, reduced-precision PE multiply at
4x the fp32 matmul rate); the residual dominates the output so the attention
path has ~20x error dilution.
"""

import sys
from contextlib import ExitStack

if "/opt/trn_rl_repo" not in sys.path:
    sys.path.insert(0, "/opt/trn_rl_repo")

import numpy as np

import concourse.bass as bass  # noqa: F401  (import keeps bass registered)
import concourse.tile as tile
from concourse import bacc, mybir
from concourse.alu_op_type import AluOpType
from concourse.bass_utils import run_bass_kernel_spmd

F32 = mybir.dt.float32
F32R = mybir.dt.float32r
AF = mybir.ActivationFunctionType
OP = AluOpType

B, C, H, W = 2, 512, 64, 64
HW = H * W          # 4096 spatial tokens
P = 128             # partitions
CT = C // P         # 4 channel tiles
NCORES = 8
QN = HW // 4        # 1024 queries per core
CHW = 512           # token chunk width
NCH = HW // CHW     # 8 chunks
JT = HW // P        # 32 key tiles
EPS = 1e-6
SCALE = float(C) ** -0.5
GPT = P // 16       # 8 groups per channel tile

USE_F32R = True


MDT = F32R if USE_F32R else F32


def _mm(ap):
    return ap


def _build_body(nc, tc, ctx, d):
    xb_d = d["xb"]
    wT_d = {n: d[n] for n in ("wqT", "wkT", "wvT", "wpT")}
    y_d = d["y"]

    cpool = ctx.enter_context(tc.tile_pool(name="const", bufs=1))
    ppool = ctx.enter_context(tc.tile_pool(name="persist", bufs=1))
    spool = ctx.enter_context(tc.tile_pool(name="stream", bufs=2))
    smpool = ctx.enter_context(tc.tile_pool(name="small", bufs=1))
    qpool = ctx.enter_context(tc.tile_pool(name="psum", bufs=2, space="PSUM"))

    # ---- constants ----
    wts = {}
    for name in ("wqT", "wkT", "wvT"):
        wts[name] = []
        for t in range(CT):
            tag = f"wkp{t}" if name == "wkT" else f"{name}{t}"
            w = cpool.tile([P, C], MDT, tag=tag, name=f"{name}{t}")
            nc.sync.dma_start(w[:], wT_d[name][t])
            wts[name].append(w)
    chv = []
    for t in range(CT):
        v = cpool.tile([P, 4], F32, tag=f"chv{t}", name=f"chv{t}")
        nc.sync.dma_start(v[:], d["chv"][t])
        chv.append(v)
    chvec = {name: [chv[t][:, i:i + 1] for t in range(CT)]
             for i, name in enumerate(("gamma", "beta", "bq", "bk"))}
    bvr = cpool.tile([1, C], MDT, tag="bvr")
    nc.sync.dma_start(bvr[:], d["bv_row"][:])
    bpr = cpool.tile([1, C], MDT, tag="bpr")
    nc.sync.dma_start(bpr[:], d["bp_row"][:])
    ind = cpool.tile([P, GPT], F32, tag="ind")
    nc.sync.dma_start(ind[:], d["ind"][:])
    indT = cpool.tile([GPT, P], F32, tag="indT")
    nc.sync.dma_start(indT[:], d["indT"][:])
    ones_col = cpool.tile([P, 1], MDT, tag="onesc")
    nc.sync.dma_start(ones_col[:], d["ones_col"][:])
    ones_row = cpool.tile([1, CHW], MDT, tag="onesr")
    nc.sync.dma_start(ones_row[:], d["ones_row"][:])
    ones_r32 = cpool.tile([1, P], F32, tag="onesr32")
    nc.vector.memset(ones_r32[:], 1.0)
    epst = cpool.tile([GPT, 1], F32, tag="eps")
    nc.vector.memset(epst[:], EPS)

    # ---- phase 1: GroupNorm statistics ----
    gps = qpool.tile([GPT, 2 * CT], F32, tag="pa")
    for t in range(CT):
        st = smpool.tile([P, NCH, 6], F32, tag="st", bufs=1)
        for ch in range(NCH):
            xt = spool.tile([P, CHW], F32, tag="sx", bufs=3)
            nc.sync.dma_start(xt[:], xb_d[t, :, ch * CHW:(ch + 1) * CHW])
            nc.vector.bn_stats(st[:, ch, :], xt[:])
        mv = smpool.tile([P, 2], F32, tag="mv", bufs=1)
        nc.vector.bn_aggr(mv[:], st[:])
        sq = smpool.tile([P, 1], F32, tag="sq", bufs=1)
        nc.vector.tensor_tensor(sq[:], mv[:, 0:1], mv[:, 0:1], op=OP.mult)
        s2 = smpool.tile([P, 2], F32, tag="s2", bufs=1)
        nc.vector.tensor_copy(s2[:, 0:1], mv[:, 0:1])
        nc.vector.tensor_tensor(s2[:, 1:2], sq[:], mv[:, 1:2], op=OP.add)
        nc.tensor.matmul(gps[:, 2 * t:2 * t + 2], ind[:], s2[:],
                         start=True, stop=True)

    gst = smpool.tile([GPT, 2 * CT], F32, tag="gst")
    nc.vector.tensor_copy(gst[:], gps[:])
    g3 = gst.rearrange("p (t two) -> p t two", two=2)
    msq = smpool.tile([GPT, CT], F32, tag="msq")
    nc.vector.tensor_tensor(msq[:], g3[:, :, 0], g3[:, :, 0], op=OP.mult)
    varg = smpool.tile([GPT, CT], F32, tag="varg")
    nc.vector.tensor_tensor(varg[:], g3[:, :, 1], msq[:], op=OP.subtract)
    stdg = smpool.tile([GPT, CT], F32, tag="stdg")
    nc.scalar.activation(stdg[:], varg[:], AF.Sqrt, bias=epst[:])
    rstdg = smpool.tile([GPT, CT], F32, tag="rstdg")
    nc.vector.reciprocal(rstdg[:], stdg[:])

    sbts = []
    for t in range(CT):
        mb = smpool.tile([GPT, 2], F32, tag="mb", bufs=1)
        nc.vector.tensor_copy(mb[:, 0:1], g3[:, t, 0:1])
        nc.vector.tensor_copy(mb[:, 1:2], rstdg[:, t:t + 1])
        cbp = qpool.tile([P, 2], F32, tag="pa")
        nc.tensor.matmul(cbp[:], indT[:], mb[:], start=True, stop=True)
        cb = smpool.tile([P, 2], F32, tag="cb", bufs=1)
        nc.vector.tensor_copy(cb[:], cbp[:])
        sbt = ppool.tile([P, 2], F32, tag=f"sb{t}")
        nc.vector.tensor_tensor(sbt[:, 0:1], cb[:, 1:2], chvec["gamma"][t][:],
                                op=OP.mult)
        tmpb = smpool.tile([P, 1], F32, tag="tmpb", bufs=1)
        nc.vector.tensor_tensor(tmpb[:], cb[:, 0:1], sbt[:, 0:1], op=OP.mult)
        nc.vector.tensor_tensor(sbt[:, 1:2], chvec["beta"][t][:], tmpb[:],
                                op=OP.subtract)
        sbts.append(sbt)

    # ---- persistent attention operands ----
    k_sb = [ppool.tile([P, HW], MDT, tag=f"k{t}", name=f"k{t}") for t in range(CT)]
    q_sb = [ppool.tile([P, QN], MDT, tag=f"q{t}", name=f"q{t}") for t in range(CT)]
    vT_sb = [ppool.tile([P, C], MDT, tag=f"vT{j}", name=f"vT{j}") for j in range(JT)]

    # ---- phase 2: q/k/v projections, streamed over token chunks ----
    for ch in range(NCH):
        sl = slice(ch * CHW, (ch + 1) * CHW)
        hns = []
        for t in range(CT):
            xt = spool.tile([P, CHW], F32, tag="sx", bufs=3)
            nc.sync.dma_start(xt[:], xb_d[t, :, sl])
            hn = spool.tile([P, CHW], MDT, tag=f"hx{t}", bufs=2)
            nc.vector.tensor_scalar(hn[:], xt[:], sbts[t][:, 0:1],
                                    sbts[t][:, 1:2], OP.mult, OP.add)
            hns.append(hn)
        for ot in range(CT):
            pk = qpool.tile([P, CHW], F32, tag="pa")
            for t in range(CT):
                nc.tensor.matmul(pk[:], _mm(wts["wkT"][t][:, ot * P:(ot + 1) * P]),
                                 _mm(hns[t][:]), start=(t == 0), stop=(t == CT - 1))
            nc.vector.tensor_scalar(k_sb[ot][:, sl], pk[:], chvec["bk"][ot][:],
                                    None, OP.add)
        for nt in range(CT):
            pv = qpool.tile([P, CHW], F32, tag="pa")
            for t in range(CT):
                nc.tensor.matmul(pv[:], _mm(hns[t][:, nt * P:(nt + 1) * P]),
                                 _mm(wts["wvT"][t][:]), start=(t == 0),
                                 stop=(t == CT - 1))
            nc.scalar.copy(vT_sb[ch * CT + nt][:], pv[:])
        if ch * CHW < QN:
            for ot in range(CT):
                pq = qpool.tile([P, CHW], F32, tag="pa")
                for t in range(CT):
                    nc.tensor.matmul(pq[:], _mm(wts["wqT"][t][:, ot * P:(ot + 1) * P]),
                                     _mm(hns[t][:]), start=(t == 0),
                                     stop=(t == CT - 1))
                nc.vector.tensor_scalar(q_sb[ot][:, sl], pq[:],
                                        chvec["bq"][ot][:], None, OP.add)

    # ---- phase 3: attention, per query half ----
    # wpT reuses wkT's slots (k_sb is already materialized, wkT is dead)
    wts["wpT"] = []
    for t in range(CT):
        w = cpool.tile([P, C], MDT, tag=f"wkp{t}", name=f"wpT{t}")
        nc.sync.dma_start(w[:], wT_d["wpT"][t])
        wts["wpT"].append(w)
    for ih in range(QN // CHW):
        isl = slice(ih * CHW, (ih + 1) * CHW)
        qs = [q_sb[t][:, isl] for t in range(CT)]
        pr = qpool.tile([1, CHW], F32, tag="pr", bufs=1)
        po = [qpool.tile([P, CHW], F32, tag=f"po{t}", name=f"po{t}", bufs=1) for t in range(CT)]
        for j in range(JT):
            ps_ = qpool.tile([P, CHW], F32, tag="pa")
            for t in range(CT):
                nc.tensor.matmul(ps_[:], _mm(k_sb[t][:, j * P:(j + 1) * P]),
                                 _mm(qs[t]), start=(t == 0), stop=(t == CT - 1))
            pT = spool.tile([P, CHW], MDT, tag="sx", bufs=3)
            nc.scalar.activation(pT[:], ps_[:], AF.Exp, scale=SCALE)
            nc.tensor.matmul(pr[:], _mm(ones_col[:]), _mm(pT[:]),
                             start=(j == 0), stop=(j == JT - 1))
            for t in range(CT):
                nc.tensor.matmul(po[t][:], _mm(vT_sb[j][:, t * P:(t + 1) * P]),
                                 _mm(pT[:]), start=(j == 0), stop=False)
        rsb = smpool.tile([1, CHW], MDT, tag="rsb", bufs=1)
        nc.vector.tensor_copy(rsb[:], pr[:])
        for t in range(CT):
            nc.tensor.matmul(po[t][:], _mm(bvr[:, t * P:(t + 1) * P]),
                             _mm(rsb[:]), start=False, stop=True)
        rr = smpool.tile([1, CHW], F32, tag="rr", bufs=1)
        nc.vector.reciprocal(rr[:], rsb[:])
        prb = qpool.tile([P, CHW], F32, tag="pa")
        nc.tensor.matmul(prb[:], ones_r32[:], rr[:], start=True, stop=True)
        rb = smpool.tile([P, CHW], F32, tag="rb", bufs=1)
        nc.vector.tensor_copy(rb[:], prb[:])
        has = []
        for t in range(CT):
            ha = spool.tile([P, CHW], MDT, tag=f"hx{t}", bufs=2)
            nc.vector.tensor_tensor(ha[:], po[t][:], rb[:], op=OP.mult)
            has.append(ha)
        for ot in range(CT):
            py = qpool.tile([P, CHW], F32, tag="pa")
            for t in range(CT):
                nc.tensor.matmul(py[:], _mm(wts["wpT"][t][:, ot * P:(ot + 1) * P]),
                                 _mm(has[t][:]), start=(t == 0), stop=False)
            nc.tensor.matmul(py[:], _mm(bpr[:, ot * P:(ot + 1) * P]),
                             _mm(ones_row[:]), start=False, stop=True)
            xr = spool.tile([P, CHW], F32, tag="sx", bufs=3)
            nc.sync.dma_start(xr[:], xb_d[ot, :, isl])
            yt = spool.tile([P, CHW], F32, tag="sx", bufs=3)
            nc.vector.tensor_tensor(yt[:], py[:], xr[:], op=OP.add)
            nc.gpsimd.dma_start(y_d[ot, :, isl], yt[:])


def build_module():
    nc = bacc.Bacc("TRN2", target_bir_lowering=False, debug=False,
                   num_devices=NCORES)
    d = {
        "xb": nc.dram_tensor("xb", [CT, P, HW], F32, kind="ExternalInput").ap(),
        "wqT": nc.dram_tensor("wqT", [CT, P, C], MDT, kind="ExternalInput").ap(),
        "wkT": nc.dram_tensor("wkT", [CT, P, C], MDT, kind="ExternalInput").ap(),
        "wvT": nc.dram_tensor("wvT", [CT, P, C], MDT, kind="ExternalInput").ap(),
        "wpT": nc.dram_tensor("wpT", [CT, P, C], MDT, kind="ExternalInput").ap(),
        "chv": nc.dram_tensor("chv", [CT, P, 4], F32, kind="ExternalInput").ap(),
        "bv_row": nc.dram_tensor("bv_row", [1, C], MDT, kind="ExternalInput").ap(),
        "bp_row": nc.dram_tensor("bp_row", [1, C], MDT, kind="ExternalInput").ap(),
        "ind": nc.dram_tensor("ind", [P, GPT], F32, kind="ExternalInput").ap(),
        "ones_col": nc.dram_tensor("ones_col", [P, 1], MDT, kind="ExternalInput").ap(),
        "ones_row": nc.dram_tensor("ones_row", [1, CHW], MDT, kind="ExternalInput").ap(),
        "indT": nc.dram_tensor("indT", [GPT, P], F32, kind="ExternalInput").ap(),
        "y": nc.dram_tensor("y", [CT, P, QN], F32, kind="ExternalOutput").ap(),
    }
    with tile.TileContext(nc) as tc, ExitStack() as ctx:
        _build_body(nc, tc, ctx, d)
    nc.compile()
    return nc


_CACHE = {}


def _get_nc():
    if "nc" not in _CACHE:
        _CACHE["nc"] = build_module()
    return _CACHE["nc"]


def _shared_inputs(gamma, beta, wq, bq, wk, bk, wv, bv, wp, bp):
    def wT(w):
        return np.ascontiguousarray(np.asarray(w, np.float32).T).reshape(CT, P, C)

    ind = np.zeros((P, GPT), np.float32)
    for i in range(P):
        ind[i, i // 16] = 1.0 / 16.0
    indT = np.zeros((GPT, P), np.float32)
    for i in range(P):
        indT[i // 16, i] = 1.0
    chv = np.stack([np.asarray(a, np.float32) for a in (gamma, beta, bq, bk)],
                   axis=1).reshape(CT, P, 4)
    return {
        "wqT": wT(wq), "wkT": wT(wk), "wvT": wT(wv), "wpT": wT(wp),
        "chv": np.ascontiguousarray(chv),
        "bv_row": np.asarray(bv, np.float32).reshape(1, C),
        "bp_row": np.asarray(bp, np.float32).reshape(1, C),
        "ind": ind, "indT": indT,
        "ones_col": np.ones((P, 1), np.float32),
        "ones_row": np.ones((1, CHW), np.float32),
    }


def make_in_maps(x, gamma, beta, wq, bq, wk, bk, wv, bv, wp, bp):
    shared = _shared_inputs(gamma, beta, wq, bq, wk, bk, wv, bv, wp, bp)
    xf = np.asarray(x, np.float32).reshape(B, C, HW)
    in_maps = []
    for core in range(NCORES):
        b, qc = divmod(core, NCORES // B)
        xb = np.roll(xf[b], -qc * QN, axis=1)
        m = dict(shared)
        m["xb"] = np.ascontiguousarray(xb).reshape(CT, P, HW)
        in_maps.append(m)
    return in_maps


def assemble_output(results):
    out = np.empty((B, C, HW), np.float32)
    for core in range(NCORES):
        b, qc = divmod(core, NCORES // B)
        y = np.asarray(results[core]["y"]).reshape(C, QN)
        out[b, :, qc * QN:(qc + 1) * QN] = y
    return out.reshape(B, C, H, W)


def kernel(x, gamma, beta, wq, bq, wk, bk, wv, bv, wp, bp):
    nc = _get_nc()
    in_maps = make_in_maps(x, gamma, beta, wq, bq, wk, bk, wv, bv, wp, bp)
    res = run_bass_kernel_spmd(nc, in_maps, list(range(NCORES)))
    return assemble_output(res.results)


# revision 14
# speedup vs baseline: 1.0266x; 1.0266x over previous
"""Trainium2 Bass kernel for nn_AttentionBlock (GroupNorm + single-head spatial
self-attention + residual) on 8 NeuronCores.

Sharding: data-parallel over batch (2) x sequence-parallel over the query
dimension (4 chunks of 1024 of the 4096 spatial tokens). Each core gets the
full image of its batch element, ROTATED so its query chunk sits at token 0
(GroupNorm stats, key/value sets and softmax sums are permutation-invariant
over tokens, so rotation lets all 8 cores run the identical SPMD program).

Per-core dataflow (all channel-major [C on partitions] except v):
  phase 1: GroupNorm stats via bn_stats/bn_aggr per channel, group-combine via
           a tiny PE matmul with a 1/16 block indicator, broadcast back with a
           second indicator matmul -> per-channel Scale/Bias.
  phase 2: stream x in 512-token chunks, hn = x*Scale+Bias, project
           k [C, 4096], vT [4096, C] (transposed layout so the later AV matmul
           needs no transposes), q [C, 1024] (first two chunks = query tokens).
  phase 3: per 512-query half: scores^T [j,128, i,512] = k_tile^T @ q (PSUM
           accum over C), exp on ScalarE straight out of PSUM (no max
           subtraction -- logits are O(5)), row-sums r via a ones-vector
           matmul, AV accum hattn[c, i] += vT_tile^T @ p without any
           transposes, deferred softmax normalization folded into the
           PSUM->SBUF move, then the output projection + bias + residual.

Matmuls run as float32r (full fp32 storage, reduced-precision PE multiply at
4x the fp32 matmul rate); the residual dominates the output so the attention
path has ~20x error dilution.
"""

import sys
from contextlib import ExitStack

if "/opt/trn_rl_repo" not in sys.path:
    sys.path.insert(0, "/opt/trn_rl_repo")

import numpy as np

import concourse.bass as bass  # noqa: F401  (import keeps bass registered)
import concourse.tile as tile
from concourse import bacc, mybir
from concourse.alu_op_type import AluOpType
from concourse.bass_utils import run_bass_kernel_spmd

F32 = mybir.dt.float32
F32R = mybir.dt.float32r
AF = mybir.ActivationFunctionType
OP = AluOpType

B, C, H, W = 2, 512, 64, 64
HW = H * W          # 4096 spatial tokens
P = 128             # partitions
CT = C // P         # 4 channel tiles
NCORES = 8
QN = HW // 4        # 1024 queries per core
CHW = 512           # token chunk width
NCH = HW // CHW     # 8 chunks
JT = HW // P        # 32 key tiles
EPS = 1e-6
SCALE = float(C) ** -0.5
GPT = P // 16       # 8 groups per channel tile

USE_F32R = True


MDT = F32R if USE_F32R else F32


def _mm(ap):
    return ap


def _build_body(nc, tc, ctx, d, with_bias):
    xb_d = d["xb"]
    wT_d = {n: d[n] for n in ("wqT", "wkT", "wvT", "wpT")}
    y_d = d["y"]

    cpool = ctx.enter_context(tc.tile_pool(name="const", bufs=1))
    ppool = ctx.enter_context(tc.tile_pool(name="persist", bufs=1))
    spool = ctx.enter_context(tc.tile_pool(name="stream", bufs=2))
    smpool = ctx.enter_context(tc.tile_pool(name="small", bufs=1))
    qpool = ctx.enter_context(tc.tile_pool(name="psum", bufs=2, space="PSUM"))

    # ---- small constants (weights load after phase 1 to give stats DMA priority) ----
    chv = []
    for t in range(CT):
        v = cpool.tile([P, 4], F32, tag=f"chv{t}", name=f"chv{t}")
        nc.sync.dma_start(v[:], d["chv"][t])
        chv.append(v)
    chvec = {name: [chv[t][:, i:i + 1] for t in range(CT)]
             for i, name in enumerate(("gamma", "beta", "bq", "bk"))}
    ind = cpool.tile([P, GPT], F32, tag="ind")
    nc.sync.dma_start(ind[:], d["ind"][:])
    indT = cpool.tile([GPT, P], F32, tag="indT")
    nc.sync.dma_start(indT[:], d["indT"][:])
    ones_col = cpool.tile([P, 1], MDT, tag="onesc")
    nc.sync.dma_start(ones_col[:], d["ones_col"][:])
    ones_row = cpool.tile([1, CHW], MDT, tag="onesr")
    nc.sync.dma_start(ones_row[:], d["ones_row"][:])
    ones_r32 = cpool.tile([1, P], F32, tag="onesr32")
    nc.vector.memset(ones_r32[:], 1.0)
    epst = cpool.tile([GPT, 1], F32, tag="eps")
    nc.vector.memset(epst[:], EPS)

    # ---- phase 1: GroupNorm statistics (4 c-tile chains interleaved) ----
    gps = qpool.tile([GPT, 2 * CT], F32, tag="pa")
    sts = [smpool.tile([P, NCH, 6], F32, tag="st", bufs=CT, name=f"st{t}")
           for t in range(CT)]
    for ch in range(NCH):
        for t in range(CT):
            xt = spool.tile([P, CHW], F32, tag="sx", bufs=3)
            nc.sync.dma_start(xt[:], xb_d[t, :, ch * CHW:(ch + 1) * CHW])
            nc.vector.bn_stats(sts[t][:, ch, :], xt[:])
    for t in range(CT):
        mv = smpool.tile([P, 2], F32, tag="mv", bufs=2)
        nc.vector.bn_aggr(mv[:], sts[t][:])
        sq = smpool.tile([P, 1], F32, tag="sq", bufs=2)
        nc.vector.tensor_tensor(sq[:], mv[:, 0:1], mv[:, 0:1], op=OP.mult)
        s2 = smpool.tile([P, 2], F32, tag="s2", bufs=2)
        nc.vector.tensor_copy(s2[:, 0:1], mv[:, 0:1])
        nc.vector.tensor_tensor(s2[:, 1:2], sq[:], mv[:, 1:2], op=OP.add)
        nc.tensor.matmul(gps[:, 2 * t:2 * t + 2], ind[:], s2[:],
                         start=True, stop=True)

    gst = smpool.tile([GPT, 2 * CT], F32, tag="gst")
    nc.vector.tensor_copy(gst[:], gps[:])
    g3 = gst.rearrange("p (t two) -> p t two", two=2)
    msq = smpool.tile([GPT, CT], F32, tag="msq")
    nc.vector.tensor_tensor(msq[:], g3[:, :, 0], g3[:, :, 0], op=OP.mult)
    varg = smpool.tile([GPT, CT], F32, tag="varg")
    nc.vector.tensor_tensor(varg[:], g3[:, :, 1], msq[:], op=OP.subtract)
    stdg = smpool.tile([GPT, CT], F32, tag="stdg")
    nc.scalar.activation(stdg[:], varg[:], AF.Sqrt, bias=epst[:])
    rstdg = smpool.tile([GPT, CT], F32, tag="rstdg")
    nc.vector.reciprocal(rstdg[:], stdg[:])

    sbts = []
    for t in range(CT):
        mb = smpool.tile([GPT, 2], F32, tag="mb", bufs=1)
        nc.vector.tensor_copy(mb[:, 0:1], g3[:, t, 0:1])
        nc.vector.tensor_copy(mb[:, 1:2], rstdg[:, t:t + 1])
        cbp = qpool.tile([P, 2], F32, tag="pa")
        nc.tensor.matmul(cbp[:], indT[:], mb[:], start=True, stop=True)
        cb = smpool.tile([P, 2], F32, tag="cb", bufs=1)
        nc.vector.tensor_copy(cb[:], cbp[:])
        sbt = ppool.tile([P, 2], F32, tag=f"sb{t}")
        nc.vector.tensor_tensor(sbt[:, 0:1], cb[:, 1:2], chvec["gamma"][t][:],
                                op=OP.mult)
        tmpb = smpool.tile([P, 1], F32, tag="tmpb", bufs=1)
        nc.vector.tensor_tensor(tmpb[:], cb[:, 0:1], sbt[:, 0:1], op=OP.mult)
        nc.vector.tensor_tensor(sbt[:, 1:2], chvec["beta"][t][:], tmpb[:],
                                op=OP.subtract)
        sbts.append(sbt)

    # ---- bulk constants: projection weights + bias rows ----
    wts = {}
    for name in ("wqT", "wkT", "wvT"):
        wts[name] = []
        for t in range(CT):
            tag = f"wkp{t}" if name == "wkT" else f"{name}{t}"
            w = cpool.tile([P, C], MDT, tag=tag, name=f"{name}{t}")
            nc.sync.dma_start(w[:], wT_d[name][t])
            wts[name].append(w)
    bvr = cpool.tile([1, C], MDT, tag="bvr")
    nc.sync.dma_start(bvr[:], d["bv_row"][:])
    bpr = cpool.tile([1, C], MDT, tag="bpr")
    nc.sync.dma_start(bpr[:], d["bp_row"][:])

    # ---- persistent attention operands ----
    k_sb = [ppool.tile([P, HW], MDT, tag=f"k{t}", name=f"k{t}") for t in range(CT)]
    q_sb = [ppool.tile([P, QN], MDT, tag=f"q{t}", name=f"q{t}") for t in range(CT)]
    vT_sb = [ppool.tile([P, C], MDT, tag=f"vT{j}", name=f"vT{j}") for j in range(JT)]

    # ---- phase 2: q/k/v projections, streamed over token chunks ----
    for ch in range(NCH):
        sl = slice(ch * CHW, (ch + 1) * CHW)
        hns = []
        for t in range(CT):
            xt = spool.tile([P, CHW], F32, tag="sx", bufs=3)
            nc.sync.dma_start(xt[:], xb_d[t, :, sl])
            hn = spool.tile([P, CHW], MDT, tag=f"hx{t}", bufs=2)
            nc.vector.tensor_scalar(hn[:], xt[:], sbts[t][:, 0:1],
                                    sbts[t][:, 1:2], OP.mult, OP.add)
            hns.append(hn)
        for ot in range(CT):
            pk = qpool.tile([P, CHW], F32, tag="pa")
            for t in range(CT):
                nc.tensor.matmul(pk[:], _mm(wts["wkT"][t][:, ot * P:(ot + 1) * P]),
                                 _mm(hns[t][:]), start=(t == 0), stop=(t == CT - 1))
            if with_bias:
                nc.vector.tensor_scalar(k_sb[ot][:, sl], pk[:],
                                        chvec["bk"][ot][:], None, OP.add)
            else:
                nc.vector.tensor_copy(k_sb[ot][:, sl], pk[:])
        for nt in range(CT):
            pv = qpool.tile([P, CHW], F32, tag="pa")
            for t in range(CT):
                nc.tensor.matmul(pv[:], _mm(hns[t][:, nt * P:(nt + 1) * P]),
                                 _mm(wts["wvT"][t][:]), start=(t == 0),
                                 stop=(t == CT - 1))
            nc.scalar.copy(vT_sb[ch * CT + nt][:], pv[:])
        if ch * CHW < QN:
            for ot in range(CT):
                pq = qpool.tile([P, CHW], F32, tag="pa")
                for t in range(CT):
                    nc.tensor.matmul(pq[:], _mm(wts["wqT"][t][:, ot * P:(ot + 1) * P]),
                                     _mm(hns[t][:]), start=(t == 0),
                                     stop=(t == CT - 1))
                if with_bias:
                    nc.vector.tensor_scalar(q_sb[ot][:, sl], pq[:],
                                            chvec["bq"][ot][:], None, OP.add)
                else:
                    nc.vector.tensor_copy(q_sb[ot][:, sl], pq[:])

    # ---- phase 3: attention, per query half ----
    # wpT reuses wkT's slots (k_sb is already materialized, wkT is dead)
    wts["wpT"] = []
    for t in range(CT):
        w = cpool.tile([P, C], MDT, tag=f"wkp{t}", name=f"wpT{t}")
        nc.sync.dma_start(w[:], wT_d["wpT"][t])
        wts["wpT"].append(w)
    for ih in range(QN // CHW):
        isl = slice(ih * CHW, (ih + 1) * CHW)
        qs = [q_sb[t][:, isl] for t in range(CT)]
        pr = qpool.tile([1, CHW], F32, tag="pr", bufs=1)
        po = [qpool.tile([P, CHW], F32, tag=f"po{t}", name=f"po{t}", bufs=1) for t in range(CT)]
        for j in range(JT):
            ps_ = qpool.tile([P, CHW], F32, tag="pa")
            for t in range(CT):
                nc.tensor.matmul(ps_[:], _mm(k_sb[t][:, j * P:(j + 1) * P]),
                                 _mm(qs[t]), start=(t == 0), stop=(t == CT - 1))
            pT = spool.tile([P, CHW], MDT, tag="sx", bufs=3)
            nc.scalar.activation(pT[:], ps_[:], AF.Exp, scale=SCALE)
            nc.tensor.matmul(pr[:], _mm(ones_col[:]), _mm(pT[:]),
                             start=(j == 0), stop=(j == JT - 1))
            for t in range(CT):
                nc.tensor.matmul(po[t][:], _mm(vT_sb[j][:, t * P:(t + 1) * P]),
                                 _mm(pT[:]), start=(j == 0),
                                 stop=(not with_bias and j == JT - 1))
        rsb = smpool.tile([1, CHW], F32, tag="rx", bufs=2)
        nc.vector.tensor_copy(rsb[:], pr[:])
        if with_bias:
            rsbr = smpool.tile([1, CHW], MDT, tag="rsbr", bufs=1)
            nc.vector.tensor_copy(rsbr[:], rsb[:])
            for t in range(CT):
                nc.tensor.matmul(po[t][:], _mm(bvr[:, t * P:(t + 1) * P]),
                                 _mm(rsbr[:]), start=False, stop=True)
        # 1/r via exp(-ln(r)) on ScalarE: much faster than DVE's iterative
        # reciprocal on a single-partition row, and off the DVE critical path
        rln = smpool.tile([1, CHW], F32, tag="rx", bufs=2)
        nc.scalar.activation(rln[:], rsb[:], AF.Ln)
        rinv = smpool.tile([1, CHW], F32, tag="rx", bufs=2)
        nc.scalar.activation(rinv[:], rln[:], AF.Exp, scale=-1.0)
        prb = qpool.tile([P, CHW], F32, tag="pa")
        nc.tensor.matmul(prb[:], ones_r32[:], rinv[:], start=True, stop=True)
        rb = smpool.tile([P, CHW], F32, tag="rb", bufs=1)
        nc.vector.tensor_copy(rb[:], prb[:])
        has = []
        for t in range(CT):
            ha = spool.tile([P, CHW], MDT, tag=f"hx{t}", bufs=2)
            nc.vector.tensor_tensor(ha[:], po[t][:], rb[:], op=OP.mult)
            has.append(ha)
        for ot in range(CT):
            py = qpool.tile([P, CHW], F32, tag="pa")
            for t in range(CT):
                nc.tensor.matmul(py[:], _mm(wts["wpT"][t][:, ot * P:(ot + 1) * P]),
                                 _mm(has[t][:]), start=(t == 0),
                                 stop=(not with_bias and t == CT - 1))
            if with_bias:
                nc.tensor.matmul(py[:], _mm(bpr[:, ot * P:(ot + 1) * P]),
                                 _mm(ones_row[:]), start=False, stop=True)
            xr = spool.tile([P, CHW], F32, tag="sx", bufs=3)
            nc.sync.dma_start(xr[:], xb_d[ot, :, isl])
            yt = spool.tile([P, CHW], F32, tag="sx", bufs=3)
            nc.vector.tensor_tensor(yt[:], py[:], xr[:], op=OP.add)
            nc.gpsimd.dma_start(y_d[ot, :, isl], yt[:])


def build_module(with_bias=True):
    nc = bacc.Bacc("TRN2", target_bir_lowering=False, debug=False,
                   num_devices=NCORES)
    d = {
        "xb": nc.dram_tensor("xb", [CT, P, HW], F32, kind="ExternalInput").ap(),
        "wqT": nc.dram_tensor("wqT", [CT, P, C], MDT, kind="ExternalInput").ap(),
        "wkT": nc.dram_tensor("wkT", [CT, P, C], MDT, kind="ExternalInput").ap(),
        "wvT": nc.dram_tensor("wvT", [CT, P, C], MDT, kind="ExternalInput").ap(),
        "wpT": nc.dram_tensor("wpT", [CT, P, C], MDT, kind="ExternalInput").ap(),
        "chv": nc.dram_tensor("chv", [CT, P, 4], F32, kind="ExternalInput").ap(),
        "bv_row": nc.dram_tensor("bv_row", [1, C], MDT, kind="ExternalInput").ap(),
        "bp_row": nc.dram_tensor("bp_row", [1, C], MDT, kind="ExternalInput").ap(),
        "ind": nc.dram_tensor("ind", [P, GPT], F32, kind="ExternalInput").ap(),
        "ones_col": nc.dram_tensor("ones_col", [P, 1], MDT, kind="ExternalInput").ap(),
        "ones_row": nc.dram_tensor("ones_row", [1, CHW], MDT, kind="ExternalInput").ap(),
        "indT": nc.dram_tensor("indT", [GPT, P], F32, kind="ExternalInput").ap(),
        "y": nc.dram_tensor("y", [CT, P, QN], F32, kind="ExternalOutput").ap(),
    }
    with tile.TileContext(nc) as tc, ExitStack() as ctx:
        _build_body(nc, tc, ctx, d, with_bias)
    nc.compile()
    return nc


_CACHE = {}


def _get_nc(with_bias=True):
    key = ("nc", with_bias)
    if key not in _CACHE:
        _CACHE[key] = build_module(with_bias)
    return _CACHE[key]


def _shared_inputs(gamma, beta, wq, bq, wk, bk, wv, bv, wp, bp):
    def wT(w):
        return np.ascontiguousarray(np.asarray(w, np.float32).T).reshape(CT, P, C)

    ind = np.zeros((P, GPT), np.float32)
    for i in range(P):
        ind[i, i // 16] = 1.0 / 16.0
    indT = np.zeros((GPT, P), np.float32)
    for i in range(P):
        indT[i // 16, i] = 1.0
    chv = np.stack([np.asarray(a, np.float32) for a in (gamma, beta, bq, bk)],
                   axis=1).reshape(CT, P, 4)
    return {
        "wqT": wT(wq), "wkT": wT(wk), "wvT": wT(wv), "wpT": wT(wp),
        "chv": np.ascontiguousarray(chv),
        "bv_row": np.asarray(bv, np.float32).reshape(1, C),
        "bp_row": np.asarray(bp, np.float32).reshape(1, C),
        "ind": ind, "indT": indT,
        "ones_col": np.ones((P, 1), np.float32),
        "ones_row": np.ones((1, CHW), np.float32),
    }


def make_in_maps(x, gamma, beta, wq, bq, wk, bk, wv, bv, wp, bp):
    shared = _shared_inputs(gamma, beta, wq, bq, wk, bk, wv, bv, wp, bp)
    xf = np.asarray(x, np.float32).reshape(B, C, HW)
    in_maps = []
    for core in range(NCORES):
        b, qc = divmod(core, NCORES // B)
        xb = np.roll(xf[b], -qc * QN, axis=1)
        m = dict(shared)
        m["xb"] = np.ascontiguousarray(xb).reshape(CT, P, HW)
        in_maps.append(m)
    return in_maps


def assemble_output(results):
    out = np.empty((B, C, HW), np.float32)
    for core in range(NCORES):
        b, qc = divmod(core, NCORES // B)
        y = np.asarray(results[core]["y"]).reshape(C, QN)
        out[b, :, qc * QN:(qc + 1) * QN] = y
    return out.reshape(B, C, H, W)


def kernel(x, gamma, beta, wq, bq, wk, bk, wv, bv, wp, bp):
    with_bias = any(np.any(np.asarray(b)) for b in (bq, bk, bv, bp))
    nc = _get_nc(with_bias)
    in_maps = make_in_maps(x, gamma, beta, wq, bq, wk, bk, wv, bv, wp, bp)
    res = run_bass_kernel_spmd(nc, in_maps, list(range(NCORES)))
    return assemble_output(res.results)


# revision 15
# speedup vs baseline: 1.0348x; 1.0080x over previous
"""Trainium2 Bass kernel for nn_AttentionBlock (GroupNorm + single-head spatial
self-attention + residual) on 8 NeuronCores.

Sharding: data-parallel over batch (2) x sequence-parallel over the query
dimension (4 chunks of 1024 of the 4096 spatial tokens). Each core gets the
full image of its batch element, ROTATED so its query chunk sits at token 0
(GroupNorm stats, key/value sets and softmax sums are permutation-invariant
over tokens, so rotation lets all 8 cores run the identical SPMD program).

Per-core dataflow (all channel-major [C on partitions] except v):
  phase 1: GroupNorm stats via bn_stats/bn_aggr per channel, group-combine via
           a tiny PE matmul with a 1/16 block indicator, broadcast back with a
           second indicator matmul -> per-channel Scale/Bias.
  phase 2: stream x in 512-token chunks, hn = x*Scale+Bias, project
           k [C, 4096], vT [4096, C] (transposed layout so the later AV matmul
           needs no transposes), q [C, 1024] (first two chunks = query tokens).
  phase 3: per 512-query half: scores^T [j,128, i,512] = k_tile^T @ q (PSUM
           accum over C), exp on ScalarE straight out of PSUM (no max
           subtraction -- logits are O(5)), row-sums r via a ones-vector
           matmul, AV accum hattn[c, i] += vT_tile^T @ p without any
           transposes, deferred softmax normalization folded into the
           PSUM->SBUF move, then the output projection + bias + residual.

Matmuls run as float32r (full fp32 storage, reduced-precision PE multiply at
4x the fp32 matmul rate); the residual dominates the output so the attention
path has ~20x error dilution.
"""

import sys
from contextlib import ExitStack

if "/opt/trn_rl_repo" not in sys.path:
    sys.path.insert(0, "/opt/trn_rl_repo")

import numpy as np

import concourse.bass as bass  # noqa: F401  (import keeps bass registered)
import concourse.tile as tile
from concourse import bacc, mybir
from concourse.alu_op_type import AluOpType
from concourse.bass_utils import run_bass_kernel_spmd

F32 = mybir.dt.float32
F32R = mybir.dt.float32r
AF = mybir.ActivationFunctionType
OP = AluOpType

B, C, H, W = 2, 512, 64, 64
HW = H * W          # 4096 spatial tokens
P = 128             # partitions
CT = C // P         # 4 channel tiles
NCORES = 8
QN = HW // 4        # 1024 queries per core
CHW = 512           # token chunk width
NCH = HW // CHW     # 8 chunks
JT = HW // P        # 32 key tiles
EPS = 1e-6
SCALE = float(C) ** -0.5
GPT = P // 16       # 8 groups per channel tile

USE_F32R = True


MDT = F32R if USE_F32R else F32


def _mm(ap):
    return ap


def _build_body(nc, tc, ctx, d, with_bias):
    xb_d = d["xb"]
    wT_d = {n: d[n] for n in ("wqT", "wkT", "wvT", "wpT")}
    y_d = d["y"]

    cpool = ctx.enter_context(tc.tile_pool(name="const", bufs=1))
    ppool = ctx.enter_context(tc.tile_pool(name="persist", bufs=1))
    spool = ctx.enter_context(tc.tile_pool(name="stream", bufs=2))
    smpool = ctx.enter_context(tc.tile_pool(name="small", bufs=1))
    qpool = ctx.enter_context(tc.tile_pool(name="psum", bufs=2, space="PSUM"))

    # ---- small constants (weights load after phase 1 to give stats DMA priority) ----
    chv = []
    for t in range(CT):
        v = cpool.tile([P, 4], F32, tag=f"chv{t}", name=f"chv{t}")
        nc.sync.dma_start(v[:], d["chv"][t])
        chv.append(v)
    chvec = {name: [chv[t][:, i:i + 1] for t in range(CT)]
             for i, name in enumerate(("gamma", "beta", "bq", "bk"))}
    ind = cpool.tile([P, GPT], F32, tag="ind")
    nc.sync.dma_start(ind[:], d["ind"][:])
    indT = cpool.tile([GPT, P], F32, tag="indT")
    nc.sync.dma_start(indT[:], d["indT"][:])
    ones_col = cpool.tile([P, 1], MDT, tag="onesc")
    nc.sync.dma_start(ones_col[:], d["ones_col"][:])
    ones_row = cpool.tile([1, CHW], MDT, tag="onesr")
    nc.sync.dma_start(ones_row[:], d["ones_row"][:])
    ones_r32 = cpool.tile([1, P], F32, tag="onesr32")
    nc.vector.memset(ones_r32[:], 1.0)
    epst = cpool.tile([GPT, 1], F32, tag="eps")
    nc.vector.memset(epst[:], EPS)

    # ---- phase 1: GroupNorm statistics (4 c-tile chains interleaved) ----
    gps = qpool.tile([GPT, 2 * CT], F32, tag="pa")
    sts = [smpool.tile([P, NCH, 6], F32, tag="st", bufs=CT, name=f"st{t}")
           for t in range(CT)]
    for ch in range(NCH):
        for t in range(CT):
            xt = spool.tile([P, CHW], F32, tag="sx", bufs=3)
            nc.sync.dma_start(xt[:], xb_d[ch, t])
            nc.vector.bn_stats(sts[t][:, ch, :], xt[:])
    for t in range(CT):
        mv = smpool.tile([P, 2], F32, tag="mv", bufs=2)
        nc.vector.bn_aggr(mv[:], sts[t][:])
        sq = smpool.tile([P, 1], F32, tag="sq", bufs=2)
        nc.vector.tensor_tensor(sq[:], mv[:, 0:1], mv[:, 0:1], op=OP.mult)
        s2 = smpool.tile([P, 2], F32, tag="s2", bufs=2)
        nc.vector.tensor_copy(s2[:, 0:1], mv[:, 0:1])
        nc.vector.tensor_tensor(s2[:, 1:2], sq[:], mv[:, 1:2], op=OP.add)
        nc.tensor.matmul(gps[:, 2 * t:2 * t + 2], ind[:], s2[:],
                         start=True, stop=True)

    gst = smpool.tile([GPT, 2 * CT], F32, tag="gst")
    nc.vector.tensor_copy(gst[:], gps[:])
    g3 = gst.rearrange("p (t two) -> p t two", two=2)
    msq = smpool.tile([GPT, CT], F32, tag="msq")
    nc.vector.tensor_tensor(msq[:], g3[:, :, 0], g3[:, :, 0], op=OP.mult)
    varg = smpool.tile([GPT, CT], F32, tag="varg")
    nc.vector.tensor_tensor(varg[:], g3[:, :, 1], msq[:], op=OP.subtract)
    stdg = smpool.tile([GPT, CT], F32, tag="stdg")
    nc.scalar.activation(stdg[:], varg[:], AF.Sqrt, bias=epst[:])
    rstdg = smpool.tile([GPT, CT], F32, tag="rstdg")
    nc.vector.reciprocal(rstdg[:], stdg[:])

    sbts = []
    for t in range(CT):
        mb = smpool.tile([GPT, 2], F32, tag="mb", bufs=1)
        nc.vector.tensor_copy(mb[:, 0:1], g3[:, t, 0:1])
        nc.vector.tensor_copy(mb[:, 1:2], rstdg[:, t:t + 1])
        cbp = qpool.tile([P, 2], F32, tag="pa")
        nc.tensor.matmul(cbp[:], indT[:], mb[:], start=True, stop=True)
        cb = smpool.tile([P, 2], F32, tag="cb", bufs=1)
        nc.vector.tensor_copy(cb[:], cbp[:])
        sbt = ppool.tile([P, 2], F32, tag=f"sb{t}")
        nc.vector.tensor_tensor(sbt[:, 0:1], cb[:, 1:2], chvec["gamma"][t][:],
                                op=OP.mult)
        tmpb = smpool.tile([P, 1], F32, tag="tmpb", bufs=1)
        nc.vector.tensor_tensor(tmpb[:], cb[:, 0:1], sbt[:, 0:1], op=OP.mult)
        nc.vector.tensor_tensor(sbt[:, 1:2], chvec["beta"][t][:], tmpb[:],
                                op=OP.subtract)
        sbts.append(sbt)

    # ---- bulk constants: projection weights + bias rows ----
    wts = {}
    for name in ("wqT", "wkT", "wvT"):
        wts[name] = []
        for t in range(CT):
            tag = f"wkp{t}" if name == "wkT" else f"{name}{t}"
            w = cpool.tile([P, C], MDT, tag=tag, name=f"{name}{t}")
            nc.sync.dma_start(w[:], wT_d[name][t])
            wts[name].append(w)
    bvr = cpool.tile([1, C], MDT, tag="bvr")
    nc.sync.dma_start(bvr[:], d["bv_row"][:])
    bpr = cpool.tile([1, C], MDT, tag="bpr")
    nc.sync.dma_start(bpr[:], d["bp_row"][:])

    # ---- persistent attention operands ----
    k_sb = [ppool.tile([P, HW], MDT, tag=f"k{t}", name=f"k{t}") for t in range(CT)]
    q_sb = [ppool.tile([P, QN], MDT, tag=f"q{t}", name=f"q{t}") for t in range(CT)]
    vT_sb = [ppool.tile([P, C], MDT, tag=f"vT{j}", name=f"vT{j}") for j in range(JT)]

    # ---- phase 2: q/k/v projections, streamed over token chunks ----
    for ch in range(NCH):
        sl = slice(ch * CHW, (ch + 1) * CHW)
        hns = []
        for t in range(CT):
            xt = spool.tile([P, CHW], F32, tag="sx", bufs=3)
            nc.sync.dma_start(xt[:], xb_d[ch, t])
            hn = spool.tile([P, CHW], MDT, tag=f"hx{t}", bufs=2)
            nc.vector.tensor_scalar(hn[:], xt[:], sbts[t][:, 0:1],
                                    sbts[t][:, 1:2], OP.mult, OP.add)
            hns.append(hn)
        for ot in range(CT):
            pk = qpool.tile([P, CHW], F32, tag="pa")
            for t in range(CT):
                nc.tensor.matmul(pk[:], _mm(wts["wkT"][t][:, ot * P:(ot + 1) * P]),
                                 _mm(hns[t][:]), start=(t == 0), stop=(t == CT - 1))
            if with_bias:
                nc.vector.tensor_scalar(k_sb[ot][:, sl], pk[:],
                                        chvec["bk"][ot][:], None, OP.add)
            else:
                nc.vector.tensor_copy(k_sb[ot][:, sl], pk[:])
        for nt in range(CT):
            pv = qpool.tile([P, CHW], F32, tag="pa")
            for t in range(CT):
                nc.tensor.matmul(pv[:], _mm(hns[t][:, nt * P:(nt + 1) * P]),
                                 _mm(wts["wvT"][t][:]), start=(t == 0),
                                 stop=(t == CT - 1))
            nc.scalar.copy(vT_sb[ch * CT + nt][:], pv[:])
        if ch * CHW < QN:
            for ot in range(CT):
                pq = qpool.tile([P, CHW], F32, tag="pa")
                for t in range(CT):
                    nc.tensor.matmul(pq[:], _mm(wts["wqT"][t][:, ot * P:(ot + 1) * P]),
                                     _mm(hns[t][:]), start=(t == 0),
                                     stop=(t == CT - 1))
                if with_bias:
                    nc.vector.tensor_scalar(q_sb[ot][:, sl], pq[:],
                                            chvec["bq"][ot][:], None, OP.add)
                else:
                    nc.vector.tensor_copy(q_sb[ot][:, sl], pq[:])

    # ---- phase 3: attention, per query half ----
    # wpT reuses wkT's slots (k_sb is already materialized, wkT is dead)
    wts["wpT"] = []
    for t in range(CT):
        w = cpool.tile([P, C], MDT, tag=f"wkp{t}", name=f"wpT{t}")
        nc.sync.dma_start(w[:], wT_d["wpT"][t])
        wts["wpT"].append(w)
    for ih in range(QN // CHW):
        isl = slice(ih * CHW, (ih + 1) * CHW)
        qs = [q_sb[t][:, isl] for t in range(CT)]
        pr = qpool.tile([1, CHW], F32, tag="pr", bufs=1)
        po = [qpool.tile([P, CHW], F32, tag=f"po{t}", name=f"po{t}", bufs=1) for t in range(CT)]
        for j in range(JT):
            ps_ = qpool.tile([P, CHW], F32, tag="pa")
            for t in range(CT):
                nc.tensor.matmul(ps_[:], _mm(k_sb[t][:, j * P:(j + 1) * P]),
                                 _mm(qs[t]), start=(t == 0), stop=(t == CT - 1))
            pT = spool.tile([P, CHW], MDT, tag="sx", bufs=3)
            nc.scalar.activation(pT[:], ps_[:], AF.Exp, scale=SCALE)
            nc.tensor.matmul(pr[:], _mm(ones_col[:]), _mm(pT[:]),
                             start=(j == 0), stop=(j == JT - 1))
            for t in range(CT):
                nc.tensor.matmul(po[t][:], _mm(vT_sb[j][:, t * P:(t + 1) * P]),
                                 _mm(pT[:]), start=(j == 0),
                                 stop=(not with_bias and j == JT - 1))
        rsb = smpool.tile([1, CHW], F32, tag="rx", bufs=2)
        nc.vector.tensor_copy(rsb[:], pr[:])
        if with_bias:
            rsbr = smpool.tile([1, CHW], MDT, tag="rsbr", bufs=1)
            nc.vector.tensor_copy(rsbr[:], rsb[:])
            for t in range(CT):
                nc.tensor.matmul(po[t][:], _mm(bvr[:, t * P:(t + 1) * P]),
                                 _mm(rsbr[:]), start=False, stop=True)
        # 1/r via exp(-ln(r)) on ScalarE: much faster than DVE's iterative
        # reciprocal on a single-partition row, and off the DVE critical path
        rln = smpool.tile([1, CHW], F32, tag="rx", bufs=2)
        nc.scalar.activation(rln[:], rsb[:], AF.Ln)
        rinv = smpool.tile([1, CHW], F32, tag="rx", bufs=2)
        nc.scalar.activation(rinv[:], rln[:], AF.Exp, scale=-1.0)
        prb = qpool.tile([P, CHW], F32, tag="pa")
        nc.tensor.matmul(prb[:], ones_r32[:], rinv[:], start=True, stop=True)
        rb = smpool.tile([P, CHW], F32, tag="rb", bufs=1)
        nc.vector.tensor_copy(rb[:], prb[:])
        has = []
        for t in range(CT):
            ha = spool.tile([P, CHW], MDT, tag=f"hx{t}", bufs=2)
            nc.vector.tensor_tensor(ha[:], po[t][:], rb[:], op=OP.mult)
            has.append(ha)
        for ot in range(CT):
            py = qpool.tile([P, CHW], F32, tag="pa")
            for t in range(CT):
                nc.tensor.matmul(py[:], _mm(wts["wpT"][t][:, ot * P:(ot + 1) * P]),
                                 _mm(has[t][:]), start=(t == 0),
                                 stop=(not with_bias and t == CT - 1))
            if with_bias:
                nc.tensor.matmul(py[:], _mm(bpr[:, ot * P:(ot + 1) * P]),
                                 _mm(ones_row[:]), start=False, stop=True)
            xr = spool.tile([P, CHW], F32, tag="sx", bufs=3)
            nc.sync.dma_start(xr[:], xb_d[ih, ot])
            yt = spool.tile([P, CHW], F32, tag="sx", bufs=3)
            nc.vector.tensor_tensor(yt[:], py[:], xr[:], op=OP.add)
            nc.gpsimd.dma_start(y_d[ot, :, isl], yt[:])


def build_module(with_bias=True):
    nc = bacc.Bacc("TRN2", target_bir_lowering=False, debug=False,
                   num_devices=NCORES)
    d = {
        "xb": nc.dram_tensor("xb", [NCH, CT, P, CHW], F32, kind="ExternalInput").ap(),
        "wqT": nc.dram_tensor("wqT", [CT, P, C], MDT, kind="ExternalInput").ap(),
        "wkT": nc.dram_tensor("wkT", [CT, P, C], MDT, kind="ExternalInput").ap(),
        "wvT": nc.dram_tensor("wvT", [CT, P, C], MDT, kind="ExternalInput").ap(),
        "wpT": nc.dram_tensor("wpT", [CT, P, C], MDT, kind="ExternalInput").ap(),
        "chv": nc.dram_tensor("chv", [CT, P, 4], F32, kind="ExternalInput").ap(),
        "bv_row": nc.dram_tensor("bv_row", [1, C], MDT, kind="ExternalInput").ap(),
        "bp_row": nc.dram_tensor("bp_row", [1, C], MDT, kind="ExternalInput").ap(),
        "ind": nc.dram_tensor("ind", [P, GPT], F32, kind="ExternalInput").ap(),
        "ones_col": nc.dram_tensor("ones_col", [P, 1], MDT, kind="ExternalInput").ap(),
        "ones_row": nc.dram_tensor("ones_row", [1, CHW], MDT, kind="ExternalInput").ap(),
        "indT": nc.dram_tensor("indT", [GPT, P], F32, kind="ExternalInput").ap(),
        "y": nc.dram_tensor("y", [CT, P, QN], F32, kind="ExternalOutput").ap(),
    }
    with tile.TileContext(nc) as tc, ExitStack() as ctx:
        _build_body(nc, tc, ctx, d, with_bias)
    nc.compile()
    return nc


_CACHE = {}


def _get_nc(with_bias=True):
    key = ("nc", with_bias)
    if key not in _CACHE:
        _CACHE[key] = build_module(with_bias)
    return _CACHE[key]


def _shared_inputs(gamma, beta, wq, bq, wk, bk, wv, bv, wp, bp):
    def wT(w):
        return np.ascontiguousarray(np.asarray(w, np.float32).T).reshape(CT, P, C)

    ind = np.zeros((P, GPT), np.float32)
    for i in range(P):
        ind[i, i // 16] = 1.0 / 16.0
    indT = np.zeros((GPT, P), np.float32)
    for i in range(P):
        indT[i // 16, i] = 1.0
    chv = np.stack([np.asarray(a, np.float32) for a in (gamma, beta, bq, bk)],
                   axis=1).reshape(CT, P, 4)
    return {
        "wqT": wT(wq), "wkT": wT(wk), "wvT": wT(wv), "wpT": wT(wp),
        "chv": np.ascontiguousarray(chv),
        "bv_row": np.asarray(bv, np.float32).reshape(1, C),
        "bp_row": np.asarray(bp, np.float32).reshape(1, C),
        "ind": ind, "indT": indT,
        "ones_col": np.ones((P, 1), np.float32),
        "ones_row": np.ones((1, CHW), np.float32),
    }


def make_in_maps(x, gamma, beta, wq, bq, wk, bk, wv, bv, wp, bp):
    shared = _shared_inputs(gamma, beta, wq, bq, wk, bk, wv, bv, wp, bp)
    xf = np.asarray(x, np.float32).reshape(B, C, HW)
    in_maps = []
    for core in range(NCORES):
        b, qc = divmod(core, NCORES // B)
        xb = np.roll(xf[b], -qc * QN, axis=1)          # [C, HW]
        xt = xb.reshape(CT, P, NCH, CHW).transpose(2, 0, 1, 3)
        m = dict(shared)
        m["xb"] = np.ascontiguousarray(xt)
        in_maps.append(m)
    return in_maps


def assemble_output(results):
    out = np.empty((B, C, HW), np.float32)
    for core in range(NCORES):
        b, qc = divmod(core, NCORES // B)
        y = np.asarray(results[core]["y"]).reshape(C, QN)
        out[b, :, qc * QN:(qc + 1) * QN] = y
    return out.reshape(B, C, H, W)


def kernel(x, gamma, beta, wq, bq, wk, bk, wv, bv, wp, bp):
    with_bias = any(np.any(np.asarray(b)) for b in (bq, bk, bv, bp))
    nc = _get_nc(with_bias)
    in_maps = make_in_maps(x, gamma, beta, wq, bq, wk, bk, wv, bv, wp, bp)
    res = run_bass_kernel_spmd(nc, in_maps, list(range(NCORES)))
    return assemble_output(res.results)
